# revision 2
# baseline (speedup 1.0000x reference)
"""Trainium2 Bass kernel: single-head attention (B=4, S=2048, D=1024) on 8 NeuronCores.

Sharding: data-parallel over (batch, query-half): core c handles batch c//2,
query rows [c%2 * 1024, (c%2+1) * 1024), full key/value sequence for its batch.

Math per core (all matmuls bf16 with fp32 PSUM accumulation):
  QT[dk,q]  = Wq(lhsT) . XqT(rhs)                 (+bq)
  KT[dk,s]  = Wk(lhsT) . XkvT(rhs)                (+bk)
  sT[s,q]   = KT-tiles(lhsT) . QT(rhs)            scores^T
  eT[s,q]   = exp(sT / sqrt(dk) [+ maskT])        on ScalarE, PSUM->SBUF bf16
  sums[1,q] = ones(lhsT) . eT(rhs)                softmax denominators
  HT[dm,q]  = Xkv-tiles(lhsT) . eT(rhs)           H = probs_unnorm @ Xkv
  out[q,dv] = HT-tiles(lhsT) . Wv(rhs)  (+bv)     (probs@Xkv)@Wv == probs@(Xkv@Wv)
  out      *= 1/sums  (per-partition scale on ScalarE, fused with PSUM->SBUF copy)

The associativity trick (out = (probs.Xkv).Wv) removes the V projection over the
full sequence entirely; the [s,q] score layout means no probs transpose is needed.
"""

import numpy as np
import ml_dtypes

B, S, D = 4, 2048, 1024
N_CORES = 8
QL = S // 2  # query rows per core (1024)
BF16 = ml_dtypes.bfloat16

_cache: dict = {}


def _build(with_mask: bool, with_bq: bool, with_bk: bool, with_bv: bool):
    import concourse.bass as bass
    import concourse.mybir as mybir
    import concourse.tile as tile
    from concourse import bacc

    fp32 = mybir.dt.float32
    bf16 = mybir.dt.bfloat16

    nc = bacc.Bacc("TRN2", target_bir_lowering=False, debug=False,
                   num_devices=N_CORES)

    # DRAM I/O (per-core shards)
    xqt_d = nc.dram_tensor("xqt", (D, QL), bf16, kind="ExternalInput")      # Xq^T
    xkvt_d = nc.dram_tensor("xkvt", (D, S), bf16, kind="ExternalInput")     # Xkv^T
    xkv_d = nc.dram_tensor("xkv", (S, D), bf16, kind="ExternalInput")       # Xkv
    wq_d = nc.dram_tensor("wq", (D, D), bf16, kind="ExternalInput")
    wk_d = nc.dram_tensor("wk", (D, D), bf16, kind="ExternalInput")
    wv_d = nc.dram_tensor("wv", (D, D), bf16, kind="ExternalInput")
    if with_bq:
        bq_d = nc.dram_tensor("bq", (128, 8), fp32, kind="ExternalInput")
    if with_bk:
        bk_d = nc.dram_tensor("bk", (128, 8), fp32, kind="ExternalInput")
    if with_bv:
        bv_d = nc.dram_tensor("bv", (1, D), bf16, kind="ExternalInput")
    if with_mask:
        maskt_d = nc.dram_tensor("maskt", (S, QL), fp32, kind="ExternalInput")
    out_d = nc.dram_tensor("out", (QL, D), fp32, kind="ExternalOutput")

    NT_D = D // 128    # 8 tiles along d_model / d_key
    NT_S = S // 128    # 16 tiles along s
    NQ = QL // 512     # 2 query chunks of 512
    NS = S // 512      # 4 s chunks of 512
    NV = D // 512      # 2 dv chunks of 512
    SCALE = 1.0 / float(np.sqrt(D))

    with tile.TileContext(nc) as tc:
        with (
            tc.tile_pool(name="cons", bufs=1) as cons,
            tc.tile_pool(name="a2", bufs=4) as a2,       # 2MB slots: wq,wk,qt,ht,wv
            tc.tile_pool(name="kt", bufs=1) as ktp,
            tc.tile_pool(name="et", bufs=1) as etp,
            tc.tile_pool(name="xq", bufs=2) as xqp,
            tc.tile_pool(name="xkvt", bufs=3) as xkvtp,
            tc.tile_pool(name="xkv", bufs=3) as xkvp,
            tc.tile_pool(name="outp", bufs=3) as outp,
            tc.tile_pool(name="mask", bufs=2) as maskp,
            tc.tile_pool(name="ps", bufs=4, space=bass.MemorySpace.PSUM) as psp,
            tc.tile_pool(name="pss", bufs=2, space=bass.MemorySpace.PSUM) as pssp,
            tc.tile_pool(name="pst", bufs=1, space=bass.MemorySpace.PSUM) as pstp,
        ):
            # ---- constants ----
            ones_col = cons.tile([128, 1], bf16, tag="ones_col")
            nc.gpsimd.memset(ones_col[:], 1.0)
            ident1 = cons.tile([1, 1], fp32, tag="ident1")
            nc.gpsimd.memset(ident1[:], 1.0)
            if with_bv:
                ones_row = cons.tile([1, 128], bf16, tag="ones_row")
                nc.gpsimd.memset(ones_row[:], 1.0)
                bv_sb = cons.tile([1, D], bf16, tag="bv")
                nc.sync.dma_start(bv_sb[:], bv_d.ap()[:])
            if with_bq:
                bq_sb = cons.tile([128, 8], fp32, tag="bq")
                nc.sync.dma_start(bq_sb[:], bq_d.ap()[:])
            if with_bk:
                bk_sb = cons.tile([128, 8], fp32, tag="bk")
                nc.sync.dma_start(bk_sb[:], bk_d.ap()[:])

            # ---- weight / activation loads (chunked for DMA parallelism) ----
            wq_sb = a2.tile([128, NT_D, D], bf16, tag="a2")
            for j in range(NT_D):
                nc.sync.dma_start(wq_sb[:, j, :], wq_d.ap()[j * 128:(j + 1) * 128, :])
            wk_sb = a2.tile([128, NT_D, D], bf16, tag="a2")
            for j in range(NT_D):
                nc.sync.dma_start(wk_sb[:, j, :], wk_d.ap()[j * 128:(j + 1) * 128, :])

            qt_sb = a2.tile([128, NT_D, QL], bf16, tag="a2")
            kt_sb = ktp.tile([128, NT_D, S], bf16, tag="kt")
            et_sb = etp.tile([128, NT_S, QL], bf16, tag="et")

            # ---- stage 1a: QT = Wq^T-free . XqT  -> qt_sb ----
            for n in range(NQ):
                xq_ch = xqp.tile([128, NT_D, 512], bf16, tag="xq")
                for j in range(NT_D):
                    nc.sync.dma_start(
                        xq_ch[:, j, :],
                        xqt_d.ap()[j * 128:(j + 1) * 128, n * 512:(n + 1) * 512])
                for i in range(NT_D):
                    ps = psp.tile([128, 512], fp32, tag="ps")
                    for j in range(NT_D):
                        nc.tensor.matmul(
                            ps[:], wq_sb[:, j, i * 128:(i + 1) * 128],
                            xq_ch[:, j, :],
                            start=(j == 0), stop=(j == NT_D - 1))
                    if with_bq:
                        nc.scalar.activation(
                            qt_sb[:, i, n * 512:(n + 1) * 512], ps[:],
                            mybir.ActivationFunctionType.Identity,
                            bias=bq_sb[:, i:i + 1])
                    else:
                        nc.scalar.activation(
                            qt_sb[:, i, n * 512:(n + 1) * 512], ps[:],
                            mybir.ActivationFunctionType.Copy)

            # ---- stage 1b: KT = Wk^T-free . XkvT -> kt_sb ----
            for n in range(NS):
                xkvt_ch = xkvtp.tile([128, NT_D, 512], bf16, tag="xkvt")
                for j in range(NT_D):
                    nc.sync.dma_start(
                        xkvt_ch[:, j, :],
                        xkvt_d.ap()[j * 128:(j + 1) * 128, n * 512:(n + 1) * 512])
                for i in range(NT_D):
                    ps = psp.tile([128, 512], fp32, tag="ps")
                    for j in range(NT_D):
                        nc.tensor.matmul(
                            ps[:], wk_sb[:, j, i * 128:(i + 1) * 128],
                            xkvt_ch[:, j, :],
                            start=(j == 0), stop=(j == NT_D - 1))
                    if with_bk:
                        nc.scalar.activation(
                            kt_sb[:, i, n * 512:(n + 1) * 512], ps[:],
                            mybir.ActivationFunctionType.Identity,
                            bias=bk_sb[:, i:i + 1])
                    else:
                        nc.scalar.activation(
                            kt_sb[:, i, n * 512:(n + 1) * 512], ps[:],
                            mybir.ActivationFunctionType.Copy)

            # ---- stage 2: scores^T, exp, and softmax denominators ----
            for n in range(NQ):
                for m in range(NT_S):
                    ps = psp.tile([128, 512], fp32, tag="ps")
                    for i in range(NT_D):
                        nc.tensor.matmul(
                            ps[:], kt_sb[:, i, m * 128:(m + 1) * 128],
                            qt_sb[:, i, n * 512:(n + 1) * 512],
                            start=(i == 0), stop=(i == NT_D - 1))
                    if with_mask:
                        mk = maskp.tile([128, 512], fp32, tag="mask")
                        nc.sync.dma_start(
                            mk[:],
                            maskt_d.ap()[m * 128:(m + 1) * 128,
                                         n * 512:(n + 1) * 512])
                        nc.vector.tensor_tensor(
                            ps[:], ps[:], mk[:], mybir.AluOpType.add)
                    nc.scalar.activation(
                        et_sb[:, m, n * 512:(n + 1) * 512], ps[:],
                        mybir.ActivationFunctionType.Exp, scale=SCALE)

            # sums[1, q] = sum_s eT  (accumulated ones-matmul)
            sums_sb = cons.tile([1, QL], fp32, tag="sums")
            for n in range(NQ):
                pss = pssp.tile([1, 512], fp32, tag="pss")
                for m in range(NT_S):
                    nc.tensor.matmul(
                        pss[:], ones_col[:], et_sb[:, m, n * 512:(n + 1) * 512],
                        start=(m == 0), stop=(m == NT_S - 1))
                nc.scalar.activation(
                    sums_sb[:, n * 512:(n + 1) * 512], pss[:],
                    mybir.ActivationFunctionType.Copy)

            # transpose sums -> [q(part), 1] and take reciprocal
            pst = pstp.tile([128, 8], fp32, tag="pst")
            for p in range(8):
                nc.tensor.transpose(
                    pst[:, p:p + 1], sums_sb[:, p * 128:(p + 1) * 128], ident1[:])
            recip_sb = cons.tile([128, 8], fp32, tag="recip")
            nc.vector.reciprocal(recip_sb[:], pst[:])

            # ---- stage 3: HT = Xkv^T-free . eT -> ht_sb ----
            ht_sb = a2.tile([128, NT_D, QL], bf16, tag="a2")
            for j in range(NT_D):
                xkv_ch = xkvp.tile([128, NT_S, 128], bf16, tag="xkv")
                nc.sync.dma_start(
                    xkv_ch[:],
                    xkv_d.ap()[:, j * 128:(j + 1) * 128]
                    .rearrange("(m p) d -> p m d", p=128))
                for n in range(NQ):
                    ps = psp.tile([128, 512], fp32, tag="ps")
                    for m in range(NT_S):
                        nc.tensor.matmul(
                            ps[:], xkv_ch[:, m, :],
                            et_sb[:, m, n * 512:(n + 1) * 512],
                            start=(m == 0), stop=(m == NT_S - 1))
                    nc.scalar.activation(
                        ht_sb[:, j, n * 512:(n + 1) * 512], ps[:],
                        mybir.ActivationFunctionType.Copy)

            # ---- stage 4: out = HT^T . Wv (+bv), normalized by 1/sums ----
            wv_sb = a2.tile([128, NT_D, D], bf16, tag="a2")
            for j in range(NT_D):
                nc.sync.dma_start(wv_sb[:, j, :], wv_d.ap()[j * 128:(j + 1) * 128, :])
            for p in range(8):
                out_sb = outp.tile([128, D], fp32, tag="outsb")
                for n2 in range(NV):
                    ps = psp.tile([128, 512], fp32, tag="ps")
                    for j in range(NT_D):
                        nc.tensor.matmul(
                            ps[:], ht_sb[:, j, p * 128:(p + 1) * 128],
                            wv_sb[:, j, n2 * 512:(n2 + 1) * 512],
                            start=(j == 0),
                            stop=(j == NT_D - 1 and not with_bv))
                    if with_bv:
                        nc.tensor.matmul(
                            ps[:], ones_row[:],
                            bv_sb[:, n2 * 512:(n2 + 1) * 512],
                            start=False, stop=True)
                    # normalize on the way out of PSUM
                    nc.scalar.activation(
                        out_sb[:, n2 * 512:(n2 + 1) * 512], ps[:],
                        mybir.ActivationFunctionType.Copy,
                        scale=recip_sb[:, p:p + 1])
                nc.sync.dma_start(
                    out_d.ap()[p * 128:(p + 1) * 128, :], out_sb[:])

    nc.compile()
    return nc


def _get_nc(flags):
    if flags not in _cache:
        _cache[flags] = _build(*flags)
    return _cache[flags]


def _prep_in_maps(query_input, keyvalue_input, mask, Wq, bq, Wk, bk, Wv, bv):
    qi = np.asarray(query_input, np.float32)
    kv = np.asarray(keyvalue_input, np.float32)
    mask = np.asarray(mask, np.float32)
    Wqb = np.asarray(Wq, np.float32).astype(BF16)
    Wkb = np.asarray(Wk, np.float32).astype(BF16)
    Wvb = np.asarray(Wv, np.float32).astype(BF16)
    bq = np.asarray(bq, np.float32)
    bk = np.asarray(bk, np.float32)
    bv = np.asarray(bv, np.float32)

    with_mask = bool(np.any(mask != 0.0))
    with_bq = bool(np.any(bq != 0.0))
    with_bk = bool(np.any(bk != 0.0))
    with_bv = bool(np.any(bv != 0.0))
    flags = (with_mask, with_bq, with_bk, with_bv)

    in_maps = []
    for c in range(N_CORES):
        b, h = c // 2, c % 2
        xq = qi[b, h * QL:(h + 1) * QL, :].astype(BF16)       # [QL, D]
        xkv_f = kv[b]                                          # [S, D] fp32
        xkv = xkv_f.astype(BF16)
        m = {
            "xqt": np.ascontiguousarray(xq.T),                 # [D, QL]
            "xkvt": np.ascontiguousarray(xkv.T),               # [D, S]
            "xkv": np.ascontiguousarray(xkv),                  # [S, D]
            "wq": Wqb, "wk": Wkb, "wv": Wvb,
        }
        if with_bq:
            m["bq"] = np.ascontiguousarray(bq.reshape(8, 128).T)
        if with_bk:
            m["bk"] = np.ascontiguousarray(bk.reshape(8, 128).T)
        if with_bv:
            m["bv"] = bv.astype(BF16).reshape(1, D)
        if with_mask:
            mt = mask[b, h * QL:(h + 1) * QL, :].T * np.float32(np.sqrt(D))
            m["maskt"] = np.ascontiguousarray(mt.astype(np.float32))
        in_maps.append(m)
    return flags, in_maps


def _run(inputs, trace=False, **kw):
    from concourse import bass_utils
    flags, in_maps = _prep_in_maps(**inputs)
    nc = _get_nc(flags)
    res = bass_utils.run_bass_kernel_spmd(
        nc, in_maps, core_ids=list(range(N_CORES)), trace=trace, **kw)
    out = np.empty((B, S, D), np.float32)
    for c in range(N_CORES):
        b, h = c // 2, c % 2
        out[b, h * QL:(h + 1) * QL, :] = res.results[c]["out"]
    return out, res


def kernel(**inputs) -> np.ndarray:
    out, _ = _run(inputs, trace=False)
    return out
